# revision 46
# baseline (speedup 1.0000x reference)
"""Trainium2 Bass kernel for PVT-style MHSA with spatial reduction.

Problem (hardcoded): B=4, C=384, H=W=64, NH=8 heads, HD=48, SR=2.
  q = Wq@x;  xsr = conv2x2s2(x, Wsr)+bsr;  k = (Wk@xsr + pos)*scale;  v = Wv@xsr
  attn = softmax(q^T k);  out = Wp@(v attn) + bp

Sharding: 8 cores = (batch b, query-half s).  Each core computes the full
conv/k/v for its batch (duplicated across the 2 cores of a batch) and
attention + projection for its 2048 queries.  No collectives.

Device notes:
  - heads padded 48 -> 64 channels; head-pair hp occupies one 128-row tile
    (head 2hp at partitions 0..47, head 2hp+1 at partitions 64..111).
  - attention computed transposed: attnT[m, n] = sum_d k[d, m] q[d, n]; the
    key-axis softmax reduction rides the AV matmul via an all-ones column in
    v^T (placed at head-local col 32 so the rowsum lands on a 32-aligned
    partition); QK^T pairs are row-packed and AV pairs col-packed in the PE
    array via tile_position.
  - all matmul operands are bf16 (host-converted; halves input DMA, enables
    FWL weight loads, lowers PE power vs float32r); accumulation is fp32 PSUM.
  - normalization: reciprocal_approx_fast on the rowsum rows, DRAM-bounce
    partition-broadcast, one tensor_tensor multiply; output is streamed out
    per query block.
  - inputs are loaded as a few large consolidated DMAs split across the two
    HWDGE rings (SP + ACT) ordered to unblock conv -> k -> first QK^T early.
"""

import threading

import ml_dtypes
import numpy as np

import concourse.bass as bass
import concourse.mybir as mybir
import concourse.tile as tile
from concourse import bacc
from concourse.bass import ts
from concourse.bass_utils import run_bass_kernel_spmd
import concourse.dve_ops as dve_ops
from concourse.dve_ops import DveOp
from concourse.dve_spec import Spec, Src0, C0, C1, sq
from concourse.dve_spec import lower as dve_lower
from concourse.dve_uop import DveOpSpec

BF16NP = ml_dtypes.bfloat16

# ---- runtime-registered custom DVE op: exp(x) ~ (c1 + c0*x)^64 -----------
# 8-slice squaring-chain approximation; c0 is tuned slightly above 1/64 to
# minimize softmax-ratio distortion over the observed logit range (+-3.9).
# Common-mode scale cancels in the softmax normalization.
EXP_A = 1.02 / 64.0


def _ref_exp_approx(in0, in1, c0, c1, c2):
    t = in0 * c0 + c1
    for _ in range(6):
        t = t * t
    return t


def _register_exp_approx():
    for op in dve_ops.OPS:
        if op.name == "EXP_APPROX_PVT":
            return op
    spec = Spec(
        body=sq(sq(sq(sq(sq(sq(Src0 * C0 + C1)))))), reference=_ref_exp_approx
    )
    row = dve_ops._CUSTOM_DVE_ROW_BASE + len(dve_ops.OPS)
    dve_ops._SUB_OPCODE_FOR_NAME["EXP_APPROX_PVT"] = row
    shas = {
        ver: DveOpSpec(
            name="EXP_APPROX_PVT", opcode=row, uops=dve_lower(spec, ver=ver),
            rd1_en=False,
        ).sha(ver)
        for ver in ("v3",)
    }
    op = DveOp("EXP_APPROX_PVT", spec, subdim=False, uops_sha=shas)
    dve_ops.OPS.append(op)
    return op


EXP_APPROX_PVT = _register_exp_approx()

# key-tiles (per group) whose exp runs on the Vector engine (custom op)
# instead of ACT, spreading the offload evenly through the pipeline
DVE_MI = (2, 5)

B, C, H, W = 4, 384, 64, 64
NH, HD, SR = 8, 48, 2
SCALE = HD ** -0.5
Hs, Ws = H // SR, W // SR
NK = Hs * Ws            # 1024 keys
N = H * W               # 4096 queries / batch
NQ = N // 2             # 2048 queries / core
CT = C // 128           # 3 c-tiles
HP = NH // 2            # 4 head-pair tiles
NB = NQ // 512          # 4 query blocks / core
MT = NK // 128          # 8 key tiles

F32 = mybir.dt.float32
F32R = mybir.dt.float32r
BF16 = mybir.dt.bfloat16
AF = mybir.ActivationFunctionType

DEFAULT_CFG = dict(qk_bufs=2, pw_bufs=2, oav_bufs=2, e_bufs=3, r_bufs=2, dr_bufs=3)


def build_program(**cfg):
    cfg = {**DEFAULT_CFG, **cfg}
    nc = bacc.Bacc(None, target_bir_lowering=False)

    xf = nc.dram_tensor("xf", [128, 2, CT, N // 2], BF16, kind="ExternalInput")
    xq = nc.dram_tensor("xq", [128, NB, CT, 512], BF16, kind="ExternalInput")
    wq = nc.dram_tensor("wq", [128, CT, 512], BF16, kind="ExternalInput")
    wk = nc.dram_tensor("wk", [128, CT, 512], BF16, kind="ExternalInput")
    wv = nc.dram_tensor("wv", [128, CT, 512], BF16, kind="ExternalInput")
    wsr = nc.dram_tensor("wsr", [128, 12, C], BF16, kind="ExternalInput")
    wp = nc.dram_tensor("wp", [128, 4, C], BF16, kind="ExternalInput")
    pos = nc.dram_tensor("pos", [128, HP, NK], F32, kind="ExternalInput")
    bsr = nc.dram_tensor("bsr", [128, CT], F32, kind="ExternalInput")
    bp = nc.dram_tensor("bp", [128, CT], F32, kind="ExternalInput")
    out = nc.dram_tensor("out", [128, CT, NQ], BF16, kind="ExternalOutput")

    with tile.TileContext(nc) as tc:
        with (
            tc.tile_pool(name="constp", bufs=1) as constp,
            tc.tile_pool(name="epool", bufs=cfg["e_bufs"]) as epool,
            tc.tile_pool(name="rpool", bufs=cfg["r_bufs"]) as rpool,
            tc.tile_pool(name="drp", bufs=cfg["dr_bufs"], space="DRAM") as drp,
            tc.tile_pool(name="ps1024", bufs=cfg["qk_bufs"], space="PSUM") as ps1024,
            tc.tile_pool(name="psw", bufs=cfg["pw_bufs"], space="PSUM") as psw,
            tc.tile_pool(name="pso", bufs=cfg["oav_bufs"], space="PSUM") as pso,
        ):
            wk_sb = constp.tile([128, CT, 512], BF16, name="wk_sb")
            wv_sb = constp.tile([128, CT, 512], BF16, name="wv_sb")
            wp_sb = constp.tile([128, 4, C], BF16, name="wp_sb")
            bsr_sb = constp.tile([128, CT], F32, name="bsr_sb")
            bp_sb = constp.tile([128, CT], F32, name="bp_sb")
            q_sb = constp.tile([128, HP, NQ], BF16, name="q_sb")
            k_sb = constp.tile([128, HP, NK], BF16, name="k_sb")
            vt_sb = constp.tile([128, MT, 512], BF16, name="vt_sb")
            xf_sb = constp.tile([128, 2, CT, N // 2], BF16, name="xf_sb")
            xq_sb = constp.tile([128, NB, CT, 512], BF16, name="xq_sb")
            wq_sb = constp.tile([128, CT, 512], BF16, name="wq_sb")
            wsr_sb = constp.tile([128, 12, C], BF16, name="wsr_sb")
            pos_sb = constp.tile([128, HP, NK], F32, name="pos_sb")
            xsr_sb = constp.tile([128, CT, NK], BF16, name="xsr_sb")
            o_sb = constp.tile([128, HP, NQ], BF16, name="o_sb")
            outp_sb = constp.tile([128, CT, NQ], BF16, name="outp_sb")

            # trigger the ACT exp table load immediately (it costs ~2.7us;
            # doing it on a scratch tile at t=0 takes it off the critical
            # path of the first real exp)
            scr = constp.tile([1, 16], F32, name="scr")
            nc.gpsimd.memset(scr[:], 0.0)
            nc.scalar.activation(out=scr[:], in_=scr[:], func=AF.Exp)

            # ACT HWDGE ring: conv weights half, early-need small tensors
            nc.scalar.dma_start(wsr_sb[:, 6:12], wsr[:, 6:12])
            nc.scalar.dma_start(pos_sb[:, 0], pos[:, 0])
            nc.scalar.dma_start(wv_sb[:], wv[:])
            for hp_ in range(1, HP):
                nc.scalar.dma_start(pos_sb[:, hp_], pos[:, hp_])
            nc.scalar.dma_start(wp_sb[:], wp[:])
            nc.scalar.dma_start(bp_sb[:], bp[:])
            # late-need bulk activations also ride the ACT ring, keeping the
            # SP ring free for the latency-critical normalize-chain DMAs
            nc.scalar.dma_start(xf_sb[:, 1], xf[:, 1])
            for nb in range(2, NB):
                nc.scalar.dma_start(xq_sb[:, nb], xq[:, nb])
            # SP HWDGE ring: first-use order; the first conv chunk (keys
            # 0:128) only reads xf rows 0:512 per c-tile, and the first
            # QK also needs wq+xq0, so those all go before the xf0 bulk
            nc.sync.dma_start(wsr_sb[:, 0:6], wsr[:, 0:6])
            nc.sync.dma_start(xf_sb[:, 0, :, 0:512], xf[:, 0, :, 0:512])
            nc.sync.dma_start(bsr_sb[:], bsr[:])
            nc.sync.dma_start(wq_sb[:], wq[:])
            nc.sync.dma_start(xq_sb[:, 0], xq[:, 0])
            nc.sync.dma_start(wk_sb[:], wk[:])
            nc.sync.dma_start(xf_sb[:, 0, :, 512:2048], xf[:, 0, :, 512:2048])
            nc.sync.dma_start(xq_sb[:, 1], xq[:, 1])

            def emit_conv(mb, k0=0, nk=512):
                """conv for keys [mb*512 + k0, mb*512 + k0 + nk); key grid is
                row-major 32x32, so a key range maps to grid rows
                [k0/32, (k0+nk)/32)."""
                for ot in range(CT):
                    p = psw.tile([128, 512], F32, name="pa", tag="pw")
                    n_mm = 0
                    for didj in range(4):
                        di, dj = didj // 2, didj % 2
                        for ci in range(CT):
                            base = xf_sb[:]
                            rhs = bass.AP(
                                tensor=base.tensor,
                                offset=base.offset
                                + mb * (CT * N // 2)
                                + ci * (N // 2)
                                + (k0 // Ws) * 2 * W
                                + di * W
                                + dj,
                                ap=[base.ap[0], [2 * W, nk // Ws], [2, Ws]],
                            )
                            nc.tensor.matmul(
                                p[:, 0:nk],
                                wsr_sb[:, didj * CT + ci, ts(ot, 128)],
                                rhs,
                                start=(n_mm == 0),
                                stop=(n_mm == 11),
                            )
                            n_mm += 1
                    nc.vector.tensor_scalar_add(
                        xsr_sb[:, ot, mb * 512 + k0 : mb * 512 + k0 + nk],
                        p[:, 0:nk],
                        bsr_sb[:, ot : ot + 1],
                    )

            def emit_k(hp, mb, k0=0, nk=512):
                p = psw.tile([128, 512], F32, name="pa", tag="pw")
                for ci in range(CT):
                    nc.tensor.matmul(
                        p[:, 0:nk],
                        wk_sb[:, ci, ts(hp, 128)],
                        xsr_sb[:, ci, mb * 512 + k0 : mb * 512 + k0 + nk],
                        start=(ci == 0),
                        stop=(ci == CT - 1),
                    )
                nc.vector.tensor_add(
                    k_sb[:, hp, mb * 512 + k0 : mb * 512 + k0 + nk],
                    p[:, 0:nk],
                    pos_sb[:, hp, mb * 512 + k0 : mb * 512 + k0 + nk],
                )

            def emit_q(ot, nb):
                p = psw.tile([128, 512], F32, name="pa", tag="pw")
                for ci in range(CT):
                    nc.tensor.matmul(
                        p[:],
                        wq_sb[:, ci, ts(ot, 128)],
                        xq_sb[:, nb, ci, :],
                        start=(ci == 0),
                        stop=(ci == CT - 1),
                    )
                nc.vector.tensor_copy(q_sb[:, ot, ts(nb, 512)], p[:])

            def emit_vt(mi):
                p = psw.tile([128, 512], F32, name="pa", tag="pw")
                for ci in range(CT):
                    nc.tensor.matmul(
                        p[:],
                        xsr_sb[:, ci, ts(mi, 128)],
                        wv_sb[:, ci, :],
                        start=(ci == 0),
                        stop=(ci == CT - 1),
                    )
                nc.vector.tensor_copy(vt_sb[:, mi, :], p[:])
                base = vt_sb[:]
                ones_ap = bass.AP(
                    tensor=base.tensor,
                    offset=base.offset + mi * 512 + 32,
                    ap=[base.ap[0], [64, NH]],
                )
                nc.gpsimd.memset(ones_ap, 1.0)

            def emit_qk_exp(nb, hp, e_sb, mis):
                for mi in mis:
                    qk = ps1024.tile([128, 1024], F32, name="qk", tag="qk")
                    nc.tensor.matmul(
                        qk[:, 0:512],
                        k_sb[0:64, hp, ts(mi, 128)],
                        q_sb[0:64, hp, ts(nb, 512)],
                        start=True,
                        stop=True,
                        tile_position=(0, 0),
                    )
                    nc.tensor.matmul(
                        qk[:, 512:1024],
                        k_sb[64:128, hp, ts(mi, 128)],
                        q_sb[64:128, hp, ts(nb, 512)],
                        start=True,
                        stop=True,
                        tile_position=(64, 0),
                    )
                    if mi in DVE_MI:
                        nc.vector._custom_dve(
                            EXP_APPROX_PVT,
                            out=e_sb[:, mi, :],
                            in0=qk[:],
                            s0=EXP_A,
                            s1=1.0,
                        )
                    else:
                        nc.scalar.activation(
                            out=e_sb[:, mi, :], in_=qk[:], func=AF.Exp
                        )

            def emit_av_norm(nb, hp, e_sb):
                oav = pso.tile([128, 512], F32, name="oav", tag="oav")
                for mi in range(MT):
                    nc.tensor.matmul(
                        oav[0:64, :],
                        vt_sb[:, mi, 128 * hp : 128 * hp + 64],
                        e_sb[:, mi, 0:512],
                        start=(mi == 0),
                        stop=(mi == MT - 1),
                        tile_position=(0, 0),
                        skip_group_check=True,
                    )
                    nc.tensor.matmul(
                        oav[64:128, :],
                        vt_sb[:, mi, 128 * hp + 64 : 128 * (hp + 1)],
                        e_sb[:, mi, 512:1024],
                        start=(mi == 0),
                        stop=(mi == MT - 1),
                        tile_position=(0, 64),
                        skip_group_check=True,
                    )
                # custom-DVE ops only run correctly at base partition 0, and
                # partition remaps are 32-aligned: gather both rowsum rows
                # into one base-0 row (column halves), then one fast recip
                g2 = rpool.tile([1, 1024], F32, name="g2", tag="g2")
                r2 = rpool.tile([1, 1024], F32, name="r2", tag="r2")
                nc.vector.tensor_copy(g2[0:1, 0:512], oav[32:33, :])
                nc.vector.tensor_copy(g2[0:1, 512:1024], oav[96:97, :])
                nc.vector.reciprocal_approx_fast(out=r2[:], in_=g2[:])
                r2d = drp.tile([1, 1024], F32, name="r2d", tag="r2d")
                nc.sync.dma_start(r2d[:], r2[:])
                rb = rpool.tile([128, 512], F32, name="rb", tag="rb")
                nc.sync.dma_start(
                    rb[0:64, :].unsqueeze(1),
                    r2d[0:1, 0:512].partition_broadcast(64),
                )
                nc.sync.dma_start(
                    rb[64:128, :].unsqueeze(1),
                    r2d[0:1, 512:1024].partition_broadcast(64),
                )
                nc.vector.tensor_mul(o_sb[:, hp, ts(nb, 512)], oav[:], rb[:])

            def emit_proj(nb):
                for ot in range(CT):
                    p = psw.tile([128, 512], F32, name="pp", tag="pw")
                    for d in range(4):
                        nc.tensor.matmul(
                            p[:],
                            wp_sb[:, d, ts(ot, 128)],
                            o_sb[:, d, ts(nb, 512)],
                            start=(d == 0),
                            stop=(d == 3),
                        )
                    nc.vector.tensor_scalar_add(
                        outp_sb[:, ot, ts(nb, 512)], p[:], bp_sb[:, ot : ot + 1]
                    )
                # stream this query block out on the ACT ring (the SP ring
                # carries the latency-critical normalize-chain DMAs)
                nc.scalar.dma_start(
                    out[:, :, ts(nb, 512)], outp_sb[:, :, ts(nb, 512)]
                )

            # interleaved, software-pipelined emission (producers strictly
            # before consumers — Tile only tracks dependencies backward):
            # chunk the first conv/k so the first exp fires early, give the
            # exp engines the first key-half of three head pairs to chew
            # during conv(1), and emit group g's QK/exp BEFORE group g-1's
            # AV/normalize so the PE prioritizes feeding the exp engines
            emit_conv(0, 0, 128)
            emit_k(0, 0, 0, 128)
            emit_q(0, 0)
            e0 = [
                epool.tile([128, MT, 1024], BF16, name="e", tag="e")
                for _ in range(3)
            ]
            emit_qk_exp(0, 0, e0[0], [0])
            emit_conv(0, 128, 384)
            emit_k(0, 0, 128, 384)
            emit_qk_exp(0, 0, e0[0], [1, 2, 3])
            emit_k(1, 0)
            emit_q(1, 0)
            emit_qk_exp(0, 1, e0[1], range(4))
            emit_k(2, 0)
            emit_q(2, 0)
            emit_qk_exp(0, 2, e0[2], range(4))
            emit_conv(1)
            emit_k(0, 1)
            emit_qk_exp(0, 0, e0[0], range(4, MT))
            emit_k(1, 1)
            emit_qk_exp(0, 1, e0[1], range(4, MT))
            for mi in range(MT):
                emit_vt(mi)

            def drain_one(pend):
                g = pend.pop(0)
                emit_av_norm(*g)
                if g[1] == HP - 1:
                    emit_proj(g[0])

            pend = [(0, 0, e0[0]), (0, 1, e0[1])]
            for nb in range(NB):
                for hp in range(HP):
                    if nb == 0 and hp < 2:
                        continue
                    if nb == 0 and hp == 2:
                        # finish the pre-started third half-group
                        emit_k(2, 1)
                        emit_qk_exp(0, 2, e0[2], range(4, MT))
                        drain_one(pend)
                        pend.append((0, 2, e0[2]))
                        continue
                    if nb == 0:
                        emit_k(hp, 0)
                        emit_k(hp, 1)
                        emit_q(hp, 0)
                    elif hp == 0:
                        for hp_ in range(HP):
                            emit_q(hp_, nb)
                    e_sb = epool.tile([128, MT, 1024], BF16, name="e", tag="e")
                    emit_qk_exp(nb, hp, e_sb, range(MT))
                    drain_one(pend)
                    pend.append((nb, hp, e_sb))
            while pend:
                drain_one(pend)

    nc.compile()
    return nc


def _pad_cols(w):
    """[C, C] weight -> [C, 512]: col 64h+j = w[48h+j, :] (j < 48)."""
    wt = np.zeros((C, NH * 64), np.float32)
    for h in range(NH):
        wt[:, 64 * h : 64 * h + HD] = w[HD * h : HD * (h + 1), :].T
    return wt


# v/proj head-local channel placement: the ones column sits at local col 32 so
# the rowsum lands on a 32-aligned PSUM partition; channel d -> col d (d<32)
# else d+1
_VCOL = np.array([d if d < 32 else d + 1 for d in range(HD)])


def _ctile(w):
    """[C, F] -> [128, CT, F] (partition-major c-tiles)."""
    return np.ascontiguousarray(w.reshape(CT, 128, -1).transpose(1, 0, 2))


def prep_inputs(inputs):
    x = np.ascontiguousarray(np.asarray(inputs["x"], np.float32))
    Wq = np.asarray(inputs["Wq"], np.float32)
    Wk = np.asarray(inputs["Wk"], np.float32)
    Wv = np.asarray(inputs["Wv"], np.float32)
    Wsr = np.asarray(inputs["Wsr"], np.float32)
    bsr = np.asarray(inputs["bsr"], np.float32)
    Wp = np.asarray(inputs["Wp"], np.float32)
    bp = np.asarray(inputs["bp"], np.float32)
    rel_h = np.asarray(inputs["rel_h"], np.float32)
    rel_w = np.asarray(inputs["rel_w"], np.float32)

    wq_t = _ctile(_pad_cols(Wq)).astype(BF16NP)
    wk_t = _ctile(_pad_cols(Wk) * SCALE).astype(BF16NP)
    wv_pad = np.zeros((C, NH * 64), np.float32)
    for h in range(NH):
        wv_pad[:, 64 * h + _VCOL] = Wv[HD * h : HD * (h + 1), :].T
    wv_t = _ctile(wv_pad).astype(BF16NP)
    # conv weights: rows ordered (di, dj, c) -> [128, 12, C] (didj, ci) tiles
    wsr_t = np.ascontiguousarray(
        Wsr.transpose(2, 3, 1, 0).reshape(12, 128, C).transpose(1, 0, 2)
    ).astype(BF16NP)
    # proj weights: row 64h + vcol(j) = Wp[:, 48h+j] -> [128, 4, C]
    wp_t = np.zeros((NH * 64, C), np.float32)
    for h in range(NH):
        wp_t[64 * h + _VCOL, :] = Wp[:, HD * h : HD * (h + 1)].T
    wp_t = np.ascontiguousarray(
        wp_t.reshape(4, 128, C).transpose(1, 0, 2)
    ).astype(BF16NP)
    # positional bias, pre-scaled, padded to 64-channel heads -> [128, HP, NK]
    pos_flat = (rel_h + rel_w).reshape(NH, HD, NK).astype(np.float32) * SCALE
    pos_t = np.zeros((NH * 64, NK), np.float32)
    for h in range(NH):
        pos_t[64 * h : 64 * h + HD, :] = pos_flat[h]
    pos_t = np.ascontiguousarray(pos_t.reshape(HP, 128, NK).transpose(1, 0, 2))
    bsr_t = np.ascontiguousarray(bsr.reshape(CT, 128).T)
    bp_t = np.ascontiguousarray(bp.reshape(CT, 128).T)

    in_maps = []
    xbf = x.astype(BF16NP)
    for core in range(8):
        b, s = core // 2, core % 2
        xb = xbf[b].reshape(C, N)
        xf_t = np.ascontiguousarray(
            xb.reshape(CT, 128, 2, N // 2).transpose(1, 2, 0, 3)
        )
        xq_t = np.ascontiguousarray(
            xb[:, s * NQ : (s + 1) * NQ]
            .reshape(CT, 128, NB, 512)
            .transpose(1, 2, 0, 3)
        )
        in_maps.append(
            {
                "xf": xf_t,
                "xq": xq_t,
                "wq": wq_t,
                "wk": wk_t,
                "wv": wv_t,
                "wsr": wsr_t,
                "wp": wp_t,
                "pos": pos_t,
                "bsr": bsr_t,
                "bp": bp_t,
            }
        )
    return in_maps


def assemble_output(results):
    out = np.empty((B, C, N), np.float32)
    for core in range(8):
        b, s = core // 2, core % 2
        out[b, :, s * NQ : (s + 1) * NQ] = (
            results[core]["out"].astype(np.float32).transpose(1, 0, 2).reshape(C, NQ)
        )
    return out.reshape(B, C, H, W)


_cache = threading.Lock()
_program = None


def get_program():
    global _program
    with _cache:
        if _program is None:
            _program = build_program()
    return _program


def run(inputs, **kwargs):
    nc = get_program()
    in_maps = prep_inputs(inputs)
    res = run_bass_kernel_spmd(nc, in_maps, core_ids=list(range(8)), **kwargs)
    return assemble_output(res.results), res


def kernel(**inputs):
    out, _ = run(inputs)
    return out



# revision 48
# speedup vs baseline: 1.1490x; 1.1490x over previous
"""Trainium2 Bass kernel for PVT-style MHSA with spatial reduction.

Problem (hardcoded): B=4, C=384, H=W=64, NH=8 heads, HD=48, SR=2.
  q = Wq@x;  xsr = conv2x2s2(x, Wsr)+bsr;  k = (Wk@xsr + pos)*scale;  v = Wv@xsr
  attn = softmax(q^T k);  out = Wp@(v attn) + bp

Sharding: 8 cores = (batch b, query-half s).  Each core computes the full
conv/k/v for its batch (duplicated across the 2 cores of a batch) and
attention + projection for its 2048 queries.  No collectives.

Device notes:
  - heads padded 48 -> 64 channels; head-pair hp occupies one 128-row tile
    (head 2hp at partitions 0..47, head 2hp+1 at partitions 64..111).
  - attention computed transposed: attnT[m, n] = sum_d k[d, m] q[d, n]; the
    key-axis softmax reduction rides the AV matmul via an all-ones column in
    v^T (placed at head-local col 32 so the rowsum lands on a 32-aligned
    partition); QK^T pairs are row-packed and AV pairs col-packed in the PE
    array via tile_position.
  - all matmul operands are bf16 (host-converted; halves input DMA, enables
    FWL weight loads, lowers PE power vs float32r); accumulation is fp32 PSUM.
  - normalization: reciprocal_approx_fast on the rowsum rows, DRAM-bounce
    partition-broadcast, one tensor_tensor multiply; output is streamed out
    per query block.
  - inputs are loaded as a few large consolidated DMAs split across the two
    HWDGE rings (SP + ACT) ordered to unblock conv -> k -> first QK^T early.
"""

import threading

import ml_dtypes
import numpy as np

import concourse.bass as bass
import concourse.mybir as mybir
import concourse.tile as tile
from concourse import bacc
from concourse.bass import ts
from concourse.bass_utils import run_bass_kernel_spmd
import concourse.dve_ops as dve_ops
from concourse.dve_ops import DveOp
from concourse.dve_spec import Spec, Src0, C0, C1, sq
from concourse.dve_spec import lower as dve_lower
from concourse.dve_uop import DveOpSpec

BF16NP = ml_dtypes.bfloat16

# ---- runtime-registered custom DVE op: exp(x) ~ (c1 + c0*x)^64 -----------
# 8-slice squaring-chain approximation; c0 is tuned slightly above 1/64 to
# minimize softmax-ratio distortion over the observed logit range (+-3.9).
# Common-mode scale cancels in the softmax normalization.
EXP_A = 1.02 / 64.0


def _ref_exp_approx(in0, in1, c0, c1, c2):
    t = in0 * c0 + c1
    for _ in range(6):
        t = t * t
    return t


def _register_exp_approx():
    for op in dve_ops.OPS:
        if op.name == "EXP_APPROX_PVT":
            return op
    spec = Spec(
        body=sq(sq(sq(sq(sq(sq(Src0 * C0 + C1)))))), reference=_ref_exp_approx
    )
    row = dve_ops._CUSTOM_DVE_ROW_BASE + len(dve_ops.OPS)
    dve_ops._SUB_OPCODE_FOR_NAME["EXP_APPROX_PVT"] = row
    shas = {
        ver: DveOpSpec(
            name="EXP_APPROX_PVT", opcode=row, uops=dve_lower(spec, ver=ver),
            rd1_en=False,
        ).sha(ver)
        for ver in ("v3",)
    }
    op = DveOp("EXP_APPROX_PVT", spec, subdim=False, uops_sha=shas)
    dve_ops.OPS.append(op)
    return op


EXP_APPROX_PVT = _register_exp_approx()

# key-tiles (per group) whose exp runs on the Vector engine (custom op)
# instead of ACT, spreading the offload evenly through the pipeline
DVE_MI = (2, 5)

B, C, H, W = 4, 384, 64, 64
NH, HD, SR = 8, 48, 2
SCALE = HD ** -0.5
Hs, Ws = H // SR, W // SR
NK = Hs * Ws            # 1024 keys
N = H * W               # 4096 queries / batch
NQ = N // 2             # 2048 queries / core
CT = C // 128           # 3 c-tiles
HP = NH // 2            # 4 head-pair tiles
NB = NQ // 512          # 4 query blocks / core
MT = NK // 128          # 8 key tiles

F32 = mybir.dt.float32
F32R = mybir.dt.float32r
BF16 = mybir.dt.bfloat16
AF = mybir.ActivationFunctionType

DEFAULT_CFG = dict(qk_bufs=2, pw_bufs=2, oav_bufs=2, e_bufs=3, r_bufs=2, dr_bufs=3)


def build_program(**cfg):
    cfg = {**DEFAULT_CFG, **cfg}
    nc = bacc.Bacc(None, target_bir_lowering=False)

    xf = nc.dram_tensor("xf", [128, 2, CT, N // 2], BF16, kind="ExternalInput")
    xq = nc.dram_tensor("xq", [128, NB, CT, 512], BF16, kind="ExternalInput")
    wq = nc.dram_tensor("wq", [128, CT, 512], BF16, kind="ExternalInput")
    wk = nc.dram_tensor("wk", [128, CT, 512], BF16, kind="ExternalInput")
    wv = nc.dram_tensor("wv", [128, CT, 512], BF16, kind="ExternalInput")
    wsr = nc.dram_tensor("wsr", [128, 12, C], BF16, kind="ExternalInput")
    wp = nc.dram_tensor("wp", [128, 4, C], BF16, kind="ExternalInput")
    pos = nc.dram_tensor("pos", [128, HP, NK], F32, kind="ExternalInput")
    bsr = nc.dram_tensor("bsr", [128, CT], F32, kind="ExternalInput")
    bp = nc.dram_tensor("bp", [128, CT], F32, kind="ExternalInput")
    out = nc.dram_tensor("out", [128, CT, NQ], BF16, kind="ExternalOutput")

    with tile.TileContext(nc) as tc:
        with (
            tc.tile_pool(name="constp", bufs=1) as constp,
            tc.tile_pool(name="epool", bufs=cfg["e_bufs"]) as epool,
            tc.tile_pool(name="rpool", bufs=cfg["r_bufs"]) as rpool,
            tc.tile_pool(name="drp", bufs=cfg["dr_bufs"], space="DRAM") as drp,
            tc.tile_pool(name="ps1024", bufs=cfg["qk_bufs"], space="PSUM") as ps1024,
            tc.tile_pool(name="psw", bufs=cfg["pw_bufs"], space="PSUM") as psw,
            tc.tile_pool(name="pso", bufs=cfg["oav_bufs"], space="PSUM") as pso,
        ):
            wk_sb = constp.tile([128, CT, 512], BF16, name="wk_sb")
            wv_sb = constp.tile([128, CT, 512], BF16, name="wv_sb")
            wp_sb = constp.tile([128, 4, C], BF16, name="wp_sb")
            bsr_sb = constp.tile([128, CT], F32, name="bsr_sb")
            bp_sb = constp.tile([128, CT], F32, name="bp_sb")
            q_sb = constp.tile([128, HP, NQ], BF16, name="q_sb")
            k_sb = constp.tile([128, HP, NK], BF16, name="k_sb")
            vt_sb = constp.tile([128, MT, 512], BF16, name="vt_sb")
            xf_sb = constp.tile([128, 2, CT, N // 2], BF16, name="xf_sb")
            xq_sb = constp.tile([128, NB, CT, 512], BF16, name="xq_sb")
            wq_sb = constp.tile([128, CT, 512], BF16, name="wq_sb")
            wsr_sb = constp.tile([128, 12, C], BF16, name="wsr_sb")
            pos_sb = constp.tile([128, HP, NK], F32, name="pos_sb")
            xsr_sb = constp.tile([128, CT, NK], BF16, name="xsr_sb")
            o_sb = constp.tile([128, HP, NQ], BF16, name="o_sb")
            outp_sb = constp.tile([128, CT, NQ], BF16, name="outp_sb")

            # trigger the ACT exp table load immediately (it costs ~2.7us;
            # doing it on a scratch tile at t=0 takes it off the critical
            # path of the first real exp)
            scr = constp.tile([1, 16], F32, name="scr")
            nc.gpsimd.memset(scr[:], 0.0)
            nc.scalar.activation(out=scr[:], in_=scr[:], func=AF.Exp)

            # ACT HWDGE ring: conv weights half, early-need small tensors
            nc.scalar.dma_start(wsr_sb[:, 6:12], wsr[:, 6:12])
            nc.scalar.dma_start(pos_sb[:, 0], pos[:, 0])
            nc.scalar.dma_start(wv_sb[:], wv[:])
            for hp_ in range(1, HP):
                nc.scalar.dma_start(pos_sb[:, hp_], pos[:, hp_])
            nc.scalar.dma_start(wp_sb[:], wp[:])
            nc.scalar.dma_start(bp_sb[:], bp[:])
            # late-need bulk activations also ride the ACT ring, keeping the
            # SP ring free for the latency-critical normalize-chain DMAs
            nc.scalar.dma_start(xf_sb[:, 1], xf[:, 1])
            for nb in range(2, NB):
                nc.scalar.dma_start(xq_sb[:, nb], xq[:, nb])
            # SP HWDGE ring: first-use order; the first conv chunk (keys
            # 0:128) only reads xf rows 0:512 per c-tile, and the first
            # QK also needs wq+xq0, so those all go before the xf0 bulk
            nc.sync.dma_start(wsr_sb[:, 0:6], wsr[:, 0:6])
            nc.sync.dma_start(xf_sb[:, 0, :, 0:512], xf[:, 0, :, 0:512])
            nc.sync.dma_start(bsr_sb[:], bsr[:])
            nc.sync.dma_start(wq_sb[:], wq[:])
            nc.sync.dma_start(xq_sb[:, 0], xq[:, 0])
            nc.sync.dma_start(wk_sb[:], wk[:])
            nc.sync.dma_start(xf_sb[:, 0, :, 512:2048], xf[:, 0, :, 512:2048])
            nc.sync.dma_start(xq_sb[:, 1], xq[:, 1])

            def emit_conv(mb, k0=0, nk=512):
                """conv for keys [mb*512 + k0, mb*512 + k0 + nk); key grid is
                row-major 32x32, so a key range maps to grid rows
                [k0/32, (k0+nk)/32)."""
                for ot in range(CT):
                    p = psw.tile([128, 512], F32, name="pa", tag="pw")
                    n_mm = 0
                    for didj in range(4):
                        di, dj = didj // 2, didj % 2
                        for ci in range(CT):
                            base = xf_sb[:]
                            rhs = bass.AP(
                                tensor=base.tensor,
                                offset=base.offset
                                + mb * (CT * N // 2)
                                + ci * (N // 2)
                                + (k0 // Ws) * 2 * W
                                + di * W
                                + dj,
                                ap=[base.ap[0], [2 * W, nk // Ws], [2, Ws]],
                            )
                            nc.tensor.matmul(
                                p[:, 0:nk],
                                wsr_sb[:, didj * CT + ci, ts(ot, 128)],
                                rhs,
                                start=(n_mm == 0),
                                stop=(n_mm == 11),
                            )
                            n_mm += 1
                    nc.vector.tensor_scalar_add(
                        xsr_sb[:, ot, mb * 512 + k0 : mb * 512 + k0 + nk],
                        p[:, 0:nk],
                        bsr_sb[:, ot : ot + 1],
                    )

            def emit_k(hp, mb, k0=0, nk=512):
                p = psw.tile([128, 512], F32, name="pa", tag="pw")
                for ci in range(CT):
                    nc.tensor.matmul(
                        p[:, 0:nk],
                        wk_sb[:, ci, ts(hp, 128)],
                        xsr_sb[:, ci, mb * 512 + k0 : mb * 512 + k0 + nk],
                        start=(ci == 0),
                        stop=(ci == CT - 1),
                    )
                nc.vector.tensor_add(
                    k_sb[:, hp, mb * 512 + k0 : mb * 512 + k0 + nk],
                    p[:, 0:nk],
                    pos_sb[:, hp, mb * 512 + k0 : mb * 512 + k0 + nk],
                )

            def emit_q(ot, nb):
                p = psw.tile([128, 512], F32, name="pa", tag="pw")
                for ci in range(CT):
                    nc.tensor.matmul(
                        p[:],
                        wq_sb[:, ci, ts(ot, 128)],
                        xq_sb[:, nb, ci, :],
                        start=(ci == 0),
                        stop=(ci == CT - 1),
                    )
                nc.vector.tensor_copy(q_sb[:, ot, ts(nb, 512)], p[:])

            def emit_vt(mi):
                p = psw.tile([128, 512], F32, name="pa", tag="pw")
                for ci in range(CT):
                    nc.tensor.matmul(
                        p[:],
                        xsr_sb[:, ci, ts(mi, 128)],
                        wv_sb[:, ci, :],
                        start=(ci == 0),
                        stop=(ci == CT - 1),
                    )
                nc.vector.tensor_copy(vt_sb[:, mi, :], p[:])
                base = vt_sb[:]
                ones_ap = bass.AP(
                    tensor=base.tensor,
                    offset=base.offset + mi * 512 + 32,
                    ap=[base.ap[0], [64, NH]],
                )
                nc.gpsimd.memset(ones_ap, 1.0)

            def emit_qk_exp(nb, hp, e_sb, mis):
                for mi in mis:
                    qk = ps1024.tile([128, 1024], F32, name="qk", tag="qk")
                    nc.tensor.matmul(
                        qk[:, 0:512],
                        k_sb[0:64, hp, ts(mi, 128)],
                        q_sb[0:64, hp, ts(nb, 512)],
                        start=True,
                        stop=True,
                        tile_position=(0, 0),
                    )
                    nc.tensor.matmul(
                        qk[:, 512:1024],
                        k_sb[64:128, hp, ts(mi, 128)],
                        q_sb[64:128, hp, ts(nb, 512)],
                        start=True,
                        stop=True,
                        tile_position=(64, 0),
                    )
                    if mi in DVE_MI:
                        nc.vector._custom_dve(
                            EXP_APPROX_PVT,
                            out=e_sb[:, mi, :],
                            in0=qk[:],
                            s0=EXP_A,
                            s1=1.0,
                        )
                    else:
                        nc.scalar.activation(
                            out=e_sb[:, mi, :], in_=qk[:], func=AF.Exp
                        )

            def emit_av_norm(nb, hp, e_sb):
                oav = pso.tile([128, 512], F32, name="oav", tag="oav")
                for mi in range(MT):
                    nc.tensor.matmul(
                        oav[0:64, :],
                        vt_sb[:, mi, 128 * hp : 128 * hp + 64],
                        e_sb[:, mi, 0:512],
                        start=(mi == 0),
                        stop=(mi == MT - 1),
                        tile_position=(0, 0),
                        skip_group_check=True,
                    )
                    nc.tensor.matmul(
                        oav[64:128, :],
                        vt_sb[:, mi, 128 * hp + 64 : 128 * (hp + 1)],
                        e_sb[:, mi, 512:1024],
                        start=(mi == 0),
                        stop=(mi == MT - 1),
                        tile_position=(0, 64),
                        skip_group_check=True,
                    )
                # custom-DVE ops only run correctly at base partition 0, and
                # partition remaps are 32-aligned: gather both rowsum rows
                # into one base-0 row (column halves), then one fast recip
                g2 = rpool.tile([1, 1024], F32, name="g2", tag="g2")
                r2 = rpool.tile([1, 1024], F32, name="r2", tag="r2")
                nc.vector.tensor_copy(g2[0:1, 0:512], oav[32:33, :])
                nc.vector.tensor_copy(g2[0:1, 512:1024], oav[96:97, :])
                nc.vector.reciprocal_approx_fast(out=r2[:], in_=g2[:])
                r2d = drp.tile([1, 1024], F32, name="r2d", tag="r2d")
                nc.sync.dma_start(r2d[:], r2[:])
                rb = rpool.tile([128, 512], F32, name="rb", tag="rb")
                nc.sync.dma_start(
                    rb[0:64, :].unsqueeze(1),
                    r2d[0:1, 0:512].partition_broadcast(64),
                )
                nc.sync.dma_start(
                    rb[64:128, :].unsqueeze(1),
                    r2d[0:1, 512:1024].partition_broadcast(64),
                )
                nc.vector.tensor_mul(o_sb[:, hp, ts(nb, 512)], oav[:], rb[:])

            def emit_proj(nb):
                for ot in range(CT):
                    p = psw.tile([128, 512], F32, name="pp", tag="pw")
                    for d in range(4):
                        nc.tensor.matmul(
                            p[:],
                            wp_sb[:, d, ts(ot, 128)],
                            o_sb[:, d, ts(nb, 512)],
                            start=(d == 0),
                            stop=(d == 3),
                        )
                    nc.vector.tensor_scalar_add(
                        outp_sb[:, ot, ts(nb, 512)], p[:], bp_sb[:, ot : ot + 1]
                    )
                # stream this query block out on the ACT ring (the SP ring
                # carries the latency-critical normalize-chain DMAs)
                nc.scalar.dma_start(
                    out[:, :, ts(nb, 512)], outp_sb[:, :, ts(nb, 512)]
                )

            # interleaved, software-pipelined emission (producers strictly
            # before consumers — Tile only tracks dependencies backward):
            # chunk the first conv/k so the first exp fires early, give the
            # exp engines the first key-half of three head pairs to chew
            # during conv(1), and emit group g's QK/exp BEFORE group g-1's
            # AV/normalize so the PE prioritizes feeding the exp engines
            emit_conv(0, 0, 128)
            emit_k(0, 0, 0, 128)
            emit_q(0, 0)
            e0 = [
                epool.tile([128, MT, 1024], BF16, name="e", tag="e")
                for _ in range(3)
            ]
            emit_qk_exp(0, 0, e0[0], [0])
            emit_conv(0, 128, 384)
            emit_k(0, 0, 128, 384)
            emit_qk_exp(0, 0, e0[0], [1, 2, 3])
            emit_k(1, 0)
            emit_q(1, 0)
            emit_qk_exp(0, 1, e0[1], range(4))
            emit_k(2, 0)
            emit_q(2, 0)
            emit_qk_exp(0, 2, e0[2], range(4))
            emit_conv(1)
            emit_k(0, 1)
            emit_qk_exp(0, 0, e0[0], range(4, MT))
            emit_k(1, 1)
            emit_qk_exp(0, 1, e0[1], range(4, MT))
            for mi in range(MT):
                emit_vt(mi)

            def drain_one(pend):
                g = pend.pop(0)
                emit_av_norm(*g)
                if g[1] == HP - 1:
                    emit_proj(g[0])

            pend = [(0, 0, e0[0]), (0, 1, e0[1])]
            for nb in range(NB):
                for hp in range(HP):
                    if nb == 0 and hp < 2:
                        continue
                    if nb == 0 and hp == 2:
                        # finish the pre-started third half-group
                        emit_k(2, 1)
                        emit_qk_exp(0, 2, e0[2], range(4, MT))
                        drain_one(pend)
                        pend.append((0, 2, e0[2]))
                        continue
                    if nb == 0:
                        emit_k(hp, 0)
                        emit_k(hp, 1)
                        emit_q(hp, 0)
                    elif hp == 0:
                        for hp_ in range(HP):
                            emit_q(hp_, nb)
                    e_sb = epool.tile([128, MT, 1024], BF16, name="e", tag="e")
                    emit_qk_exp(nb, hp, e_sb, range(MT))
                    drain_one(pend)
                    pend.append((nb, hp, e_sb))
            while pend:
                drain_one(pend)

    nc.compile()
    return nc


def _pad_cols(w):
    """[C, C] weight -> [C, 512]: col 64h+j = w[48h+j, :] (j < 48)."""
    wt = np.zeros((C, NH * 64), np.float32)
    for h in range(NH):
        wt[:, 64 * h : 64 * h + HD] = w[HD * h : HD * (h + 1), :].T
    return wt


# v/proj head-local channel placement: the ones column sits at local col 32 so
# the rowsum lands on a 32-aligned PSUM partition; channel d -> col d (d<32)
# else d+1
_VCOL = np.array([d if d < 32 else d + 1 for d in range(HD)])


def _ctile(w):
    """[C, F] -> [128, CT, F] (partition-major c-tiles)."""
    return np.ascontiguousarray(w.reshape(CT, 128, -1).transpose(1, 0, 2))


def prep_inputs(inputs):
    x = np.ascontiguousarray(np.asarray(inputs["x"], np.float32))
    Wq = np.asarray(inputs["Wq"], np.float32)
    Wk = np.asarray(inputs["Wk"], np.float32)
    Wv = np.asarray(inputs["Wv"], np.float32)
    Wsr = np.asarray(inputs["Wsr"], np.float32)
    bsr = np.asarray(inputs["bsr"], np.float32)
    Wp = np.asarray(inputs["Wp"], np.float32)
    bp = np.asarray(inputs["bp"], np.float32)
    rel_h = np.asarray(inputs["rel_h"], np.float32)
    rel_w = np.asarray(inputs["rel_w"], np.float32)

    wq_t = _ctile(_pad_cols(Wq)).astype(BF16NP)
    wk_t = _ctile(_pad_cols(Wk) * SCALE).astype(BF16NP)
    wv_pad = np.zeros((C, NH * 64), np.float32)
    for h in range(NH):
        wv_pad[:, 64 * h + _VCOL] = Wv[HD * h : HD * (h + 1), :].T
    wv_t = _ctile(wv_pad).astype(BF16NP)
    # conv weights: rows ordered (di, dj, c) -> [128, 12, C] (didj, ci) tiles
    wsr_t = np.ascontiguousarray(
        Wsr.transpose(2, 3, 1, 0).reshape(12, 128, C).transpose(1, 0, 2)
    ).astype(BF16NP)
    # proj weights: row 64h + vcol(j) = Wp[:, 48h+j] -> [128, 4, C]
    wp_t = np.zeros((NH * 64, C), np.float32)
    for h in range(NH):
        wp_t[64 * h + _VCOL, :] = Wp[:, HD * h : HD * (h + 1)].T
    wp_t = np.ascontiguousarray(
        wp_t.reshape(4, 128, C).transpose(1, 0, 2)
    ).astype(BF16NP)
    # positional bias, pre-scaled, padded to 64-channel heads -> [128, HP, NK]
    pos_flat = (rel_h + rel_w).reshape(NH, HD, NK).astype(np.float32) * SCALE
    pos_t = np.zeros((NH * 64, NK), np.float32)
    for h in range(NH):
        pos_t[64 * h : 64 * h + HD, :] = pos_flat[h]
    pos_t = np.ascontiguousarray(pos_t.reshape(HP, 128, NK).transpose(1, 0, 2))
    bsr_t = np.ascontiguousarray(bsr.reshape(CT, 128).T)
    bp_t = np.ascontiguousarray(bp.reshape(CT, 128).T)

    in_maps = []
    xbf = x.astype(BF16NP)
    for core in range(8):
        b, s = core // 2, core % 2
        xb = xbf[b].reshape(C, N)
        xf_t = np.ascontiguousarray(
            xb.reshape(CT, 128, 2, N // 2).transpose(1, 2, 0, 3)
        )
        xq_t = np.ascontiguousarray(
            xb[:, s * NQ : (s + 1) * NQ]
            .reshape(CT, 128, NB, 512)
            .transpose(1, 2, 0, 3)
        )
        in_maps.append(
            {
                "xf": xf_t,
                "xq": xq_t,
                "wq": wq_t,
                "wk": wk_t,
                "wv": wv_t,
                "wsr": wsr_t,
                "wp": wp_t,
                "pos": pos_t,
                "bsr": bsr_t,
                "bp": bp_t,
            }
        )
    return in_maps


def assemble_output(results):
    out = np.empty((B, C, N), np.float32)
    for core in range(8):
        b, s = core // 2, core % 2
        out[b, :, s * NQ : (s + 1) * NQ] = (
            results[core]["out"].astype(np.float32).transpose(1, 0, 2).reshape(C, NQ)
        )
    return out.reshape(B, C, H, W)


_cache = threading.Lock()
_program = None


def get_program():
    global _program
    with _cache:
        if _program is None:
            _program = build_program()
    return _program


def run(inputs, **kwargs):
    nc = get_program()
    in_maps = prep_inputs(inputs)
    res = run_bass_kernel_spmd(nc, in_maps, core_ids=list(range(8)), **kwargs)
    return assemble_output(res.results), res


def kernel(**inputs):
    out, _ = run(inputs)
    return out

